# revision 1
# baseline (speedup 1.0000x reference)
"""ChunkAwareAttention Trainium2 kernel (bf16 datapath).

Model (hardcoded): B=4, T=2048, D=512, H=8, DK=64, CHUNK=64, EPS=1e-5.
  xn = LayerNorm(x) * ln_w + ln_b
  q/k/v = heads(xn @ W{q,k,v} + b)        [B,H,T,DK]
  pk    = heads(pos_enc @ Wpos)[0]        [H,T,DK]
  scores = (q @ (k + pk)^T) / sqrt(DK)    (pos term folded into k)
  chunk-causal mask (attend own chunk + all previous), softmax, @v,
  out = concat_heads @ Wout + bout

Sharding over 8 cores: core c -> batch b = c//2, head-group g = c%2
(4 heads = 256 features per core). Each core computes a partial
(its heads' contribution) of out[b] @ Wout; host sums the two
partials per batch and adds bout.

Key design points:
  - all matmul operands bf16 (full-rate PE at any moving size, cheap
    LDWEIGHTS, half SBUF/DMA traffic); PSUM accumulation stays fp32.
  - pos projection (input-only) computed on HOST, added during the
    k-projection PSUM evacuation.
  - chunk-diagonal masking via a rank-1 matmul accumulate (-1e30 row x
    ones) into the score PSUM; exp(-1e30*scale) == 0.
  - LayerNorm/transposes in bf16; xnT is a mono-tile so each LN tile
    evacuates PSUM->SBUF with one strided DVE copy.
  - batched input DMAs (the SP queue serializes at ~0.6us/DMA).
  - emission interleaves LN/projections/out-projection into the
    attention stream to keep the PE p-state high; softmax norms are
    deferred, in 512-column halves (columns [g, g+512) stop
    accumulating at ki=4*qj+7, so their norm runs early and the PSUM
    bank frees sooner).
"""

import sys

if "/opt/trn_rl_repo" not in sys.path:
    sys.path.insert(0, "/opt/trn_rl_repo")

import math
import numpy as np
import ml_dtypes

import concourse.bass as bass
import concourse.tile as tile
from concourse import bacc, mybir
from concourse.bass_utils import run_bass_kernel_spmd
from concourse.masks import make_identity

B, T, D, H = 4, 2048, 512, 8
DK = D // H
CHUNK = 64
EPS = 1e-5
NCORES = 8
HPC = H // 2          # heads per core = 4
F = HPC * DK          # features per core = 256
KD = D // 128         # contraction tiles over D = 4
NT = T // 128         # 128-row tiles over T = 16
F32 = mybir.dt.float32
BF16 = mybir.dt.bfloat16
NEG = -1e30
SCALE = 1.0 / math.sqrt(DK)
LAG = 2


def _build_program():
    nc = bacc.Bacc(
        "TRN2",
        target_bir_lowering=False,
        debug=False,
        enable_asserts=False,
        num_devices=NCORES,
    )

    x_d = nc.dram_tensor("x", [T, D], BF16, kind="ExternalInput").ap()
    wq_d = nc.dram_tensor("wq", [D, F], BF16, kind="ExternalInput").ap()
    wk_d = nc.dram_tensor("wk", [D, F], BF16, kind="ExternalInput").ap()
    wv_d = nc.dram_tensor("wv", [D, F], BF16, kind="ExternalInput").ap()
    wout_d = nc.dram_tensor("wout", [F, D], BF16, kind="ExternalInput").ap()
    pkT_d = nc.dram_tensor("pkT", [F, T], BF16, kind="ExternalInput").ap()
    bq_d = nc.dram_tensor("bq", [F, 1], F32, kind="ExternalInput").ap()
    bk_d = nc.dram_tensor("bk", [F, 1], F32, kind="ExternalInput").ap()
    bv_d = nc.dram_tensor("bv", [1, F], F32, kind="ExternalInput").ap()
    out_d = nc.dram_tensor("out", [T, D], F32, kind="ExternalOutput").ap()

    with tile.TileContext(nc) as tc:
        _emit(nc, tc, x_d, wq_d, wk_d, wv_d, wout_d, pkT_d,
              bq_d, bk_d, bv_d, out_d)

    nc.compile()
    return nc


def _emit(nc, tc, x_d, wq_d, wk_d, wv_d, wout_d, pkT_d,
          bq_d, bk_d, bv_d, out_d):
    from contextlib import ExitStack

    ctx = ExitStack()
    with ctx:
        singles = ctx.enter_context(tc.tile_pool(name="singles", bufs=1))
        xnpool = ctx.enter_context(tc.tile_pool(name="xn", bufs=3))
        stats = ctx.enter_context(tc.tile_pool(name="stats", bufs=4))
        exppool = ctx.enter_context(tc.tile_pool(name="exp", bufs=4))
        rcpool = ctx.enter_context(tc.tile_pool(name="rc", bufs=2))
        opool = ctx.enter_context(tc.tile_pool(name="ostage", bufs=3))
        # PSUM: ps 2 x [128,1024]f32 (4 banks) + oacc 2 x (4 banks) = 16KB.
        ps = ctx.enter_context(tc.tile_pool(name="ps", bufs=2, space="PSUM"))
        oaccp = ctx.enter_context(
            tc.tile_pool(name="oacc", bufs=2, space="PSUM"))

        # ---- input DMAs (batched; x first: LN is the critical-path start) ----
        x_sb = singles.tile([128, NT * D], BF16, tag="xsb", name="x_sb")
        ti0 = 0
        for nti in (2, 2, 4, 4, 4):
            src = bass.AP(tensor=x_d.tensor,
                          offset=x_d.offset + ti0 * 128 * D,
                          ap=[[D, 128], [128 * D, nti], [1, D]])
            dst = x_sb[:, ti0 * D:(ti0 + nti) * D].rearrange(
                "p (t c) -> p t c", c=D)
            nc.sync.dma_start(out=dst, in_=src)
            ti0 += nti
        xts = [x_sb[:, ti * D:(ti + 1) * D] for ti in range(NT)]

        ident = singles.tile([128, 128], BF16)
        make_identity(nc, ident)
        eps_t = singles.tile([128, 1], F32)
        nc.vector.memset(eps_t, EPS)
        # rank-1 mask ingredients: row0 of maskneg = [0]*64 + [-1e30]*64
        maskneg = singles.tile([128, 128], BF16, tag="maskneg")
        nc.vector.memset(maskneg[0:1, 0:DK], 0.0)
        nc.vector.memset(maskneg[0:1, DK:128], NEG)
        onesrow = singles.tile([128, DK], BF16, tag="onesrow")
        nc.vector.memset(onesrow[0:1, :], 1.0)

        # ---- resident weights (one batched DMA per tensor) ----
        def load_mono(dram, nblk, width, nm):
            t = singles.tile([128, nblk * width], BF16, tag=nm, name=nm)
            src = bass.AP(tensor=dram.tensor, offset=dram.offset,
                          ap=[[width, 128], [128 * width, nblk], [1, width]])
            dst = t[:].rearrange("p (b c) -> p b c", c=width)
            nc.sync.dma_start(out=dst, in_=src)
            return t
        wq_t = load_mono(wq_d, KD, F, "wqall")
        wk_t = load_mono(wk_d, KD, F, "wkall")
        wv_t = load_mono(wv_d, KD, F, "wvall")
        wq_sb = [wq_t[:, kd * F:(kd + 1) * F] for kd in range(KD)]
        wk_sb = [wk_t[:, kd * F:(kd + 1) * F] for kd in range(KD)]
        wv_sb = [wv_t[:, kd * F:(kd + 1) * F] for kd in range(KD)]
        pkT_t = load_mono(pkT_d, 2, T, "pkTall")
        pkT_sb = [pkT_t[:, m * T:(m + 1) * T] for m in range(2)]
        wout_t = load_mono(wout_d, 2, D, "woutall")
        wout_sb = [wout_t[:, m * D:(m + 1) * D] for m in range(2)]
        bq_t = singles.tile([128, 2], F32, tag="bqall", name="bq_t")
        nc.sync.dma_start(out=bq_t[:], in_=bass.AP(
            tensor=bq_d.tensor, offset=bq_d.offset, ap=[[1, 128], [128, 2]]))
        bk_t = singles.tile([128, 2], F32, tag="bkall", name="bk_t")
        nc.sync.dma_start(out=bk_t[:], in_=bass.AP(
            tensor=bk_d.tensor, offset=bk_d.offset, ap=[[1, 128], [128, 2]]))
        bq_sb = [bq_t[:, m:m + 1] for m in range(2)]
        bk_sb = [bk_t[:, m:m + 1] for m in range(2)]
        bv_sb = singles.tile([128, F], F32)
        nc.gpsimd.dma_start(
            out=bv_sb[:],
            in_=bass.AP(tensor=bv_d.tensor, offset=bv_d.offset,
                        ap=[[0, 128], [1, F]]))

        # ---- big resident activations (bf16) ----
        # xnT mono-tile [128, KD*T]: slice kd at columns [kd*T, (kd+1)*T).
        xnT = singles.tile([128, KD * T], BF16, tag="xnT", name="xnT")
        qT = [singles.tile([128, T], BF16, tag=f"qT{m}", name=f"qT{m}")
              for m in range(2)]
        kT = [singles.tile([128, T], BF16, tag=f"kT{m}", name=f"kT{m}")
              for m in range(2)]
        # V natural layout, per head [V_h(64) | ones(64)]: ones columns give
        # the softmax denominator on PSUM rows 64:128 of the attnv matmul.
        v_sb = [singles.tile([128, HPC * (2 * DK)], BF16, tag=f"v{ti}",
                             name=f"v{ti}") for ti in range(NT)]
        for ti in range(NT):
            nc.gpsimd.memset(v_sb[ti][:], 1.0)  # pre-fill ones columns
        att = [singles.tile([128, T], BF16, tag=f"att{m}", name=f"att{m}")
               for m in range(2)]

        def xnT_c(kd, lo, hi):
            return xnT[:, kd * T + lo:kd * T + hi]

        # ====== LayerNorm + transpose (one 128-row tile) ======
        def emit_ln(ti):
            xt = xts[ti]
            st = stats.tile([128, 6], F32)
            nc.vector.bn_stats(out=st[:], in_=xt[:])
            mv = stats.tile([128, 2], F32)
            nc.vector.bn_aggr(out=mv[:], in_=st[:])
            rstd = stats.tile([128, 1], F32)
            nc.scalar.activation(
                out=rstd[:], in_=mv[:, 1:2],
                func=mybir.ActivationFunctionType.Sqrt,
                bias=eps_t[:], scale=1.0)
            nc.vector.reciprocal(out=rstd[:], in_=rstd[:])
            xnt = xnpool.tile([128, D], BF16)
            nc.vector.tensor_scalar(
                out=xnt[:], in0=xt[:],
                scalar1=mv[:, 0:1], scalar2=rstd[:],
                op0=mybir.AluOpType.subtract, op1=mybir.AluOpType.mult)
            pt = ps.tile([128, D], BF16, tag="ps")
            for kd in range(KD):
                nc.tensor.transpose(
                    pt[:, kd * 128:(kd + 1) * 128],
                    xnt[:, kd * 128:(kd + 1) * 128], ident[:])
            # single strided evacuation: pt[:, kd*128:+128] -> xnT col-block
            dst = bass.AP(
                tensor=xnT.tensor, offset=xnT.offset + ti * 128,
                ap=[[xnT.ap[0][0], 128], [T, KD], [1, 128]])
            src = pt[:].rearrange("p (kd c) -> p kd c", kd=KD)
            nc.vector.tensor_copy(out=dst, in_=src)

        # ====== projections ======
        def emit_proj(m, tcn):
            tsl = slice(tcn * 512, (tcn + 1) * 512)
            msl = slice(m * 128, (m + 1) * 128)
            pq = ps.tile([128, 512], F32, tag="ps")
            for kd in range(KD):
                nc.tensor.matmul(
                    pq[:], wq_sb[kd][:, msl],
                    xnT_c(kd, tcn * 512, tcn * 512 + 512),
                    start=(kd == 0), stop=(kd == KD - 1))
            nc.vector.tensor_scalar_add(
                out=qT[m][:, tsl], in0=pq[:], scalar1=bq_sb[m])
            pk = ps.tile([128, 512], F32, tag="ps")
            for kd in range(KD):
                nc.tensor.matmul(
                    pk[:], wk_sb[kd][:, msl],
                    xnT_c(kd, tcn * 512, tcn * 512 + 512),
                    start=(kd == 0), stop=(kd == KD - 1))
            # kT = (pk + bk) + pkT  (host-computed pos projection)
            nc.vector.scalar_tensor_tensor(
                out=kT[m][:, tsl], in0=pk[:], scalar=bk_sb[m],
                in1=pkT_sb[m][:, tsl],
                op0=mybir.AluOpType.add, op1=mybir.AluOpType.add)

        def emit_v(ti):
            pv = ps.tile([128, F], F32, tag="ps")
            for kd in range(KD):
                nc.tensor.matmul(
                    pv[:], xnT_c(kd, ti * 128, ti * 128 + 128), wv_sb[kd],
                    start=(kd == 0), stop=(kd == KD - 1))
            vt = v_sb[ti]
            dst = vt[:].rearrange("p (h c) -> p h c", h=HPC)[:, :, 0:DK]
            srcv = pv[:].rearrange("p (h c) -> p h c", c=DK)
            bvb = bv_sb[:].rearrange("p (h c) -> p h c", c=DK)
            nc.vector.tensor_tensor(
                out=dst, in0=srcv, in1=bvb, op=mybir.AluOpType.add)

        # ====== attention (one head, one 1024-wide q block) ======
        def emit_att(h, qj):
            m = h // 2
            r0 = DK * (h % 2)
            g = qj * 1024
            kmax = 8 * qj + 8

            def emit_scores(ki):
                qoff = 128 * ki
                kst = kT[m][r0:r0 + DK, qoff:qoff + 128]
                cs = max(qoff, g)
                ce = g + 1024
                diag = cs == qoff
                spt = ps.tile([128, 1024], F32, tag="ps", name="spt")
                for bb in range(cs // 512, (ce - 1) // 512 + 1):
                    s5 = max(cs, bb * 512)
                    e5 = min(ce, (bb + 1) * 512)
                    nc.tensor.matmul(
                        spt[:, s5 - g:e5 - g],
                        kst, qT[m][r0:r0 + DK, s5:e5],
                        start=True, stop=not (diag and s5 == cs))
                if diag:
                    # mask keys of chunk 2ki+1 vs queries of chunk 2ki:
                    # accumulate -1e30 onto rows 64:128, cols [cs, cs+64).
                    nc.tensor.matmul(
                        spt[:, cs - g:cs - g + DK],
                        maskneg[0:1, :], onesrow[0:1, :],
                        start=False, stop=True, skip_group_check=True)
                et = exppool.tile([128, 1024], BF16, tag="et", name="et")
                nc.scalar.activation(
                    out=et[:, cs - g:1024], in_=spt[:, cs - g:1024],
                    func=mybir.ActivationFunctionType.Exp,
                    scale=SCALE)
                return et

            def emit_attnv(ki, et):
                qoff = 128 * ki
                vst = v_sb[ki][:, (h % HPC) * 2 * DK:((h % HPC) + 1) * 2 * DK]
                cs = max(qoff, g)
                ce = g + 1024
                for bb in range(cs // 512, (ce - 1) // 512 + 1):
                    s5 = max(cs, bb * 512)
                    e5 = min(ce, (bb + 1) * 512)
                    last_ki = min(4 * bb + 3, kmax - 1)
                    nc.tensor.matmul(
                        out_acc[:, s5 - g:e5 - g],
                        vst, et[:, s5 - g:e5 - g],
                        start=(ki == 0), stop=(ki == last_ki))

            out_acc = oaccp.tile([128, 1024], F32, tag="oacc", name="oacc")
            pend = []
            for ki in range(kmax):
                pend.append((ki, emit_scores(ki)))
                if len(pend) > LAG:
                    k0, e0 = pend.pop(0)
                    emit_attnv(k0, e0)
            for k0, e0 in pend:
                emit_attnv(k0, e0)
            return (h, qj, out_acc)

        # deferred softmax normalization, one 512-col half at a time
        # (half A of a q block stops accumulating at ki=4*qj+7, so its
        # norm can run while the block's tail k-tiles are in flight).
        def emit_norm(state, half):
            h, qj, out_acc = state
            m = h // 2
            r0 = DK * (h % 2)
            g = qj * 1024
            c0 = half * 512
            dn = rcpool.tile([DK, 512], F32, tag="dn", name="dn")
            nc.vector.tensor_copy(
                out=dn[:], in_=out_acc[DK:2 * DK, c0:c0 + 512])
            rc = rcpool.tile([DK, 512], F32, tag="rc", name="rc")
            nc.vector.reciprocal_approx_fast(out=rc[:], in_=dn[:])
            nc.vector.tensor_tensor(
                out=att[m][r0:r0 + DK, g + c0:g + c0 + 512],
                in0=out_acc[0:DK, c0:c0 + 512],
                in1=rc[:], op=mybir.AluOpType.mult)

        def emit_norm2(state):
            emit_norm(state, 0)
            emit_norm(state, 1)

        # ====== output projection ======
        def emit_outproj(qj, half=None, ps_only=False):
            tis = range(qj * 8, qj * 8 + 8)
            if half is not None:
                tis = list(tis)[half * 4:half * 4 + 4]
            for i, ti in enumerate(tis):
                use_ps = ps_only or i % 2 == 0
                pool = ps if use_ps else oaccp
                po = pool.tile([128, 512], F32,
                               tag="ps" if use_ps else "oacc", name="po")
                for m2 in range(2):
                    nc.tensor.matmul(
                        po[:], att[m2][:, ti * 128:(ti + 1) * 128],
                        wout_sb[m2],
                        start=(m2 == 0), stop=(m2 == 1))
                og = opool.tile([128, 512], F32, tag="og", name="og")
                nc.vector.tensor_copy(out=og[:], in_=po[:])
                nc.gpsimd.dma_start(
                    out=out_d[ti * 128:(ti + 1) * 128, :], in_=og[:])

        # ====== emission schedule ======
        emit_ln(0); emit_ln(1); emit_ln(2)
        emit_v(0)
        emit_ln(3); emit_v(1)
        emit_ln(4); emit_v(2)
        emit_ln(5); emit_v(3)
        emit_ln(6); emit_v(4)
        emit_ln(7); emit_v(5)
        emit_proj(0, 0)
        emit_v(6)
        emit_proj(0, 1)
        emit_v(7)
        a00 = emit_att(0, 0)
        emit_ln(8); emit_ln(9)
        emit_ln(10); emit_v(8)
        emit_ln(11); emit_v(9)
        emit_ln(12); emit_v(10)
        emit_ln(13); emit_v(11)
        emit_ln(14); emit_v(12)
        emit_ln(15); emit_v(13)
        emit_proj(0, 2); emit_proj(0, 3)
        emit_norm2(a00)
        a10 = emit_att(1, 0)
        emit_v(14); emit_v(15)
        emit_proj(1, 0); emit_proj(1, 1)
        emit_norm2(a10)
        emit_proj(1, 2); emit_proj(1, 3)
        a20 = emit_att(2, 0)
        emit_norm2(a20)
        a30 = emit_att(3, 0)
        emit_norm2(a30)
        a01 = emit_att(0, 1)
        emit_outproj(0, half=0, ps_only=True)
        emit_norm2(a01)
        a11 = emit_att(1, 1)
        emit_outproj(0, half=1, ps_only=True)
        emit_norm2(a11)
        a21 = emit_att(2, 1)
        emit_norm2(a21)
        a31 = emit_att(3, 1)
        emit_norm2(a31)
        emit_outproj(1)


_CACHED_NC = None


def _get_nc():
    global _CACHED_NC
    if _CACHED_NC is None:
        _CACHED_NC = _build_program()
    return _CACHED_NC


def make_in_maps(x, pos_enc, mask, ln_w, ln_b, Wq, bq, Wk, bk, Wv, bv,
                 Wpos, Wout, bout):
    f32, bf = np.float32, ml_dtypes.bfloat16
    x = np.asarray(x, f32)
    pos_enc = np.asarray(pos_enc, f32)
    ln_w = np.asarray(ln_w, f32)
    ln_b = np.asarray(ln_b, f32)
    Wq, bq = np.asarray(Wq, f32), np.asarray(bq, f32)
    Wk, bk = np.asarray(Wk, f32), np.asarray(bk, f32)
    Wv, bv = np.asarray(Wv, f32), np.asarray(bv, f32)
    Wpos = np.asarray(Wpos, f32)
    Wout = np.asarray(Wout, f32)

    # Fold the LayerNorm affine into the projections (exact rewrite).
    lw = ln_w[:, None]
    Wq_f, bq_f = Wq * lw, bq + ln_b @ Wq
    Wk_f, bk_f = Wk * lw, bk + ln_b @ Wk
    Wv_f, bv_f = Wv * lw, bv + ln_b @ Wv

    # Host-side pos projection: pk = pos_enc[0] @ Wpos, shipped transposed.
    pk_full = pos_enc[0] @ Wpos  # [T, D]

    in_maps = []
    for c in range(NCORES):
        b, g = divmod(c, 2)
        hs = slice(g * F, (g + 1) * F)
        in_maps.append({
            "x": np.ascontiguousarray(x[b]).astype(bf),
            "wq": np.ascontiguousarray(Wq_f[:, hs]).astype(bf),
            "wk": np.ascontiguousarray(Wk_f[:, hs]).astype(bf),
            "wv": np.ascontiguousarray(Wv_f[:, hs]).astype(bf),
            "wout": np.ascontiguousarray(Wout[hs, :]).astype(bf),
            "pkT": np.ascontiguousarray(pk_full[:, hs].T).astype(bf),
            "bq": np.ascontiguousarray(bq_f[hs, None]),
            "bk": np.ascontiguousarray(bk_f[hs, None]),
            "bv": np.ascontiguousarray(bv_f[None, hs]),
        })
    return in_maps


def kernel(**inputs):
    in_maps = make_in_maps(**inputs)
    bout = np.asarray(inputs["bout"], np.float32)
    nc = _get_nc()
    res = run_bass_kernel_spmd(nc, in_maps, core_ids=list(range(NCORES)))

    out = np.empty((B, T, D), np.float32)
    for b in range(B):
        out[b] = res.results[2 * b]["out"] + res.results[2 * b + 1]["out"] + bout
    return out



# revision 13
# speedup vs baseline: 1.3366x; 1.3366x over previous
"""ChunkAwareAttention Trainium2 kernel (bf16 datapath), v2.

Model (hardcoded): B=4, T=2048, D=512, H=8, DK=64, CHUNK=64, EPS=1e-5.
  xn = LayerNorm(x) * ln_w + ln_b          (affine folded into W on host)
  q/k/v = heads(xn @ W{q,k,v} + b)         [B,H,T,DK]
  scores = (q @ (k + pk)^T) / sqrt(DK)     (pos term + bk folded into kT)
  chunk-causal mask, softmax, @v, out = concat_heads @ Wout + bout

Sharding over 8 cores: core c -> batch b = c//2, head-group g = c%2
(4 heads = 256 features per core). Host sums the two partials per batch.

v2 design (vs v1):
  - x shipped TRANSPOSED (tb-major [4][512d][512t]); LayerNorm stats via
    PE ones-matmuls producing partition-replicated sums, so no PE
    transposes and no bn_stats chain.  Normalize runs on GpSimd.
  - scores for the two heads of an m-group are row-packed: head A uses
    PE rows 0:63, head B rows 64:127 (tile_position auto-derived from
    base_partition) -> the two matmuls run concurrently.
  - score PSUM tile [128, 1024] = headA 512 q-cols | headB 512 q-cols
    (different banks); ONE exp per k-tile covers both heads.
  - chunk-diagonal masking via GpSimd memset-0 on the exp'd tile
    (replaces rank-1 -1e30 matmuls on PE).
  - softmax denominators via ones-columns in v (free: matmul cost is
    moving-column-bound); reciprocal reads PSUM directly.
  - wavefront schedule: attention q-blocks of 512 per m-group; LN
    normalize / projections / v / out-projection interleave into the
    ACT-bound attention stream to keep PE dense (HAM stays warm).
  - all sqrt before the first exp -> exactly 2 ACT table loads.
"""

import sys

if "/opt/trn_rl_repo" not in sys.path:
    sys.path.insert(0, "/opt/trn_rl_repo")

import math
import numpy as np
import ml_dtypes

import concourse.bass as bass
import concourse.tile as tile
from concourse import bacc, mybir
from concourse.bass_utils import run_bass_kernel_spmd

B, T, D, H = 4, 2048, 512, 8
DK = D // H
CHUNK = 64
EPS = 1e-5
NCORES = 8
HPC = H // 2          # heads per core = 4
F = HPC * DK          # features per core = 256
KD = D // 128         # contraction tiles over D = 4
NT = T // 128         # 128-row tiles over T = 16
F32 = mybir.dt.float32
BF16 = mybir.dt.bfloat16
SCALE = 1.0 / math.sqrt(DK)
LAG = 2
W = 512               # attention q-block width
NQB = T // W          # q-blocks = 4


def _build_program():
    nc = bacc.Bacc(
        "TRN2",
        target_bir_lowering=False,
        debug=False,
        enable_asserts=False,
        num_devices=NCORES,
    )

    # xt: tb-major transposed x: [4 tb][512 d][512 t] flattened.
    xt_d = nc.dram_tensor("xt", [T, D], BF16, kind="ExternalInput").ap()
    wq_d = nc.dram_tensor("wq", [D, F], BF16, kind="ExternalInput").ap()
    wk_d = nc.dram_tensor("wk", [D, F], BF16, kind="ExternalInput").ap()
    wv_d = nc.dram_tensor("wv", [D, F], BF16, kind="ExternalInput").ap()
    wout_d = nc.dram_tensor("wout", [F, D], BF16, kind="ExternalInput").ap()
    pkT_d = nc.dram_tensor("pkT", [F, T], BF16, kind="ExternalInput").ap()
    bq_d = nc.dram_tensor("bq", [F, 1], F32, kind="ExternalInput").ap()
    bv_d = nc.dram_tensor("bv", [1, F], F32, kind="ExternalInput").ap()
    out_d = nc.dram_tensor("out", [T, D], F32, kind="ExternalOutput").ap()

    with tile.TileContext(nc) as tc:
        _emit(nc, tc, xt_d, wq_d, wk_d, wv_d, wout_d, pkT_d,
              bq_d, bv_d, out_d)

    nc.compile()
    return nc


def _emit(nc, tc, xt_d, wq_d, wk_d, wv_d, wout_d, pkT_d, bq_d, bv_d, out_d):
    from contextlib import ExitStack

    ctx = ExitStack()
    with ctx:
        singles = ctx.enter_context(tc.tile_pool(name="singles", bufs=1))
        xsqpool = ctx.enter_context(tc.tile_pool(name="xsq", bufs=2))
        stpool = ctx.enter_context(tc.tile_pool(name="st", bufs=4))
        t1pool = ctx.enter_context(tc.tile_pool(name="t1", bufs=3))
        exppool = ctx.enter_context(tc.tile_pool(name="exp", bufs=4))
        rcpool = ctx.enter_context(tc.tile_pool(name="rc", bufs=4))
        opool = ctx.enter_context(tc.tile_pool(name="ostage", bufs=3))
        # PSUM: ps 2 x [128,1024] (4 banks) + oacc 2 x [128,512] (2) +
        # free 2 x [128,512] (2) = 8 banks.
        ps = ctx.enter_context(tc.tile_pool(name="ps", bufs=2, space="PSUM"))
        oaccp = ctx.enter_context(
            tc.tile_pool(name="oacc", bufs=2, space="PSUM"))
        freep = ctx.enter_context(
            tc.tile_pool(name="free", bufs=2, space="PSUM"))

        # ---- input DMAs (xt on sync queue, tb0 first; weights on gpsimd) --
        xtb = []
        for tb in range(NQB):
            t = singles.tile([128, KD * W], BF16, tag=f"xt{tb}",
                             name=f"xt{tb}")
            src = bass.AP(tensor=xt_d.tensor,
                          offset=xt_d.offset + tb * W * D,
                          ap=[[W, 128], [128 * W, KD], [1, W]])
            dst = t[:].rearrange("p (b c) -> p b c", c=W)
            nc.sync.dma_start(out=dst, in_=src)
            xtb.append(t)

        def load_mono(dram, nblk, width, nm):
            t = singles.tile([128, nblk * width], BF16, tag=nm, name=nm)
            src = bass.AP(tensor=dram.tensor, offset=dram.offset,
                          ap=[[width, 128], [128 * width, nblk], [1, width]])
            dst = t[:].rearrange("p (b c) -> p b c", c=width)
            nc.gpsimd.dma_start(out=dst, in_=src)
            return t
        wq_t = load_mono(wq_d, KD, F, "wqall")
        wk_t = load_mono(wk_d, KD, F, "wkall")
        wv_t = load_mono(wv_d, KD, F, "wvall")
        wq_sb = [wq_t[:, kd * F:(kd + 1) * F] for kd in range(KD)]
        wk_sb = [wk_t[:, kd * F:(kd + 1) * F] for kd in range(KD)]
        wv_sb = [wv_t[:, kd * F:(kd + 1) * F] for kd in range(KD)]
        pkT_t = load_mono(pkT_d, 2, T, "pkTall")
        pkT_sb = [pkT_t[:, m * T:(m + 1) * T] for m in range(2)]
        wout_t = load_mono(wout_d, 2, D, "woutall")
        wout_sb = [wout_t[:, m * D:(m + 1) * D] for m in range(2)]
        bq_t = singles.tile([128, 2], F32, tag="bqall", name="bq_t")
        nc.sync.dma_start(out=bq_t[:], in_=bass.AP(
            tensor=bq_d.tensor, offset=bq_d.offset, ap=[[1, 128], [128, 2]]))
        bq_sb = [bq_t[:, m:m + 1] for m in range(2)]
        bv_sb = singles.tile([128, F], F32)
        nc.gpsimd.dma_start(
            out=bv_sb[:],
            in_=bass.AP(tensor=bv_d.tensor, offset=bv_d.offset,
                        ap=[[0, 128], [1, F]]))

        # ---- consts ----
        ones_t = singles.tile([128, 128], BF16, tag="ones")
        nc.vector.memset(ones_t[:], 1.0)
        eps_t = singles.tile([128, 1], F32)
        nc.vector.memset(eps_t, EPS)

        # ---- stats staging (partition-replicated) ----
        mu_all = singles.tile([128, T], BF16, tag="mu", name="mu_all")
        rstd_all = singles.tile([128, T], BF16, tag="rstd", name="rstd_all")
        murstd_all = singles.tile([128, T], BF16, tag="murstd",
                                  name="murstd_all")

        # ---- big resident activations (bf16) ----
        xnT = singles.tile([128, KD * T], BF16, tag="xnT", name="xnT")
        qT = [singles.tile([128, T], BF16, tag=f"qT{m}", name=f"qT{m}")
              for m in range(2)]
        kT = [singles.tile([128, T], BF16, tag=f"kT{m}", name=f"kT{m}")
              for m in range(2)]
        # V natural layout, per head [V_h(64) | ones(64)]: ones columns give
        # the softmax denominator on PSUM rows 64:128 of the attnv matmul.
        v_sb = [singles.tile([128, HPC * (2 * DK)], BF16, tag=f"v{ti}",
                             name=f"v{ti}") for ti in range(NT)]
        for ti in range(NT):
            nc.gpsimd.memset(v_sb[ti][:], 1.0)  # pre-fill ones columns
        att = [singles.tile([128, T], BF16, tag=f"att{m}", name=f"att{m}")
               for m in range(2)]

        def xnT_c(kd, lo, hi):
            return xnT[:, kd * T + lo:kd * T + hi]

        # ====== LayerNorm stats for one 512-col t-block ======
        def emit_stats(tb):
            xsq = xsqpool.tile([128, KD * W], BF16)
            nc.scalar.square(out=xsq[:], in_=xtb[tb][:])
            psum = freep.tile([128, W], F32, tag="free", name="ps_sum")
            pssq = freep.tile([128, W], F32, tag="free", name="ps_sq")
            for kd in range(KD):
                nc.tensor.matmul(
                    psum[:], ones_t[:], xtb[tb][:, kd * W:(kd + 1) * W],
                    start=(kd == 0), stop=(kd == KD - 1))
            for kd in range(KD):
                nc.tensor.matmul(
                    pssq[:], ones_t[:], xsq[:, kd * W:(kd + 1) * W],
                    start=(kd == 0), stop=(kd == KD - 1))
            tsl = slice(tb * W, (tb + 1) * W)
            nc.vector.tensor_scalar_mul(
                out=mu_all[:, tsl], in0=psum[:], scalar1=1.0 / D)
            msq = stpool.tile([128, W], F32)
            nc.vector.tensor_scalar_mul(
                out=msq[:], in0=pssq[:], scalar1=1.0 / D)
            mumu = stpool.tile([128, W], F32)
            nc.vector.tensor_tensor(
                out=mumu[:], in0=mu_all[:, tsl], in1=mu_all[:, tsl],
                op=mybir.AluOpType.mult)
            var = stpool.tile([128, W], F32)
            nc.vector.tensor_tensor(
                out=var[:], in0=msq[:], in1=mumu[:],
                op=mybir.AluOpType.subtract)
            sd = stpool.tile([128, W], F32)
            nc.scalar.activation(
                out=sd[:], in_=var[:],
                func=mybir.ActivationFunctionType.Sqrt, bias=eps_t[:],
                scale=1.0)
            rstdf = stpool.tile([128, W], F32)
            nc.vector.reciprocal_approx_fast(out=rstdf[:], in_=sd[:])
            nc.vector.tensor_copy(out=rstd_all[:, tsl], in_=rstdf[:])
            nc.vector.tensor_tensor(
                out=murstd_all[:, tsl], in0=mu_all[:, tsl],
                in1=rstd_all[:, tsl], op=mybir.AluOpType.mult)

        # ====== normalize one (kd, tcn) tile (DVE, bf16 2x mode) ======
        def emit_norm_x(tcn, kd):
            tsl = slice(tcn * W, (tcn + 1) * W)
            t1 = t1pool.tile([128, W], BF16)
            nc.vector.tensor_tensor(
                out=t1[:], in0=xtb[tcn][:, kd * W:(kd + 1) * W],
                in1=rstd_all[:, tsl], op=mybir.AluOpType.mult)
            nc.vector.tensor_tensor(
                out=xnT_c(kd, tcn * W, tcn * W + W), in0=t1[:],
                in1=murstd_all[:, tsl], op=mybir.AluOpType.subtract)

        # ====== q/k projections for (m, tcn) ======
        def emit_proj(m, tcn):
            tsl = slice(tcn * W, (tcn + 1) * W)
            msl = slice(m * 128, (m + 1) * 128)
            pq = freep.tile([128, W], F32, tag="free", name="pq")
            for kd in range(KD):
                nc.tensor.matmul(
                    pq[:], wq_sb[kd][:, msl], xnT_c(kd, tcn * W, tcn * W + W),
                    start=(kd == 0), stop=(kd == KD - 1))
            nc.vector.tensor_scalar_add(
                out=qT[m][:, tsl], in0=pq[:], scalar1=bq_sb[m])
            pk = freep.tile([128, W], F32, tag="free", name="pk")
            for kd in range(KD):
                nc.tensor.matmul(
                    pk[:], wk_sb[kd][:, msl], xnT_c(kd, tcn * W, tcn * W + W),
                    start=(kd == 0), stop=(kd == KD - 1))
            # kT = pk + pkT  (host pos projection, bk folded in)
            nc.vector.tensor_tensor(
                out=kT[m][:, tsl], in0=pk[:], in1=pkT_sb[m][:, tsl],
                op=mybir.AluOpType.add)

        # ====== v projection for one 128-row tile ======
        def emit_v(ti):
            pv = freep.tile([128, F], F32, tag="free", name="pv")
            for kd in range(KD):
                nc.tensor.matmul(
                    pv[:], xnT_c(kd, ti * 128, ti * 128 + 128), wv_sb[kd],
                    start=(kd == 0), stop=(kd == KD - 1))
            vt = v_sb[ti]
            dst = vt[:].rearrange("p (h c) -> p h c", h=HPC)[:, :, 0:DK]
            srcv = pv[:].rearrange("p (h c) -> p h c", c=DK)
            bvb = bv_sb[:].rearrange("p (h c) -> p h c", c=DK)
            nc.vector.tensor_tensor(
                out=dst, in0=srcv, in1=bvb, op=mybir.AluOpType.add)

        # ====== attention for (m-group, q-block), heads row-packed ======
        def emit_att(m, qb, inject=None):
            kn = 4 * qb + 4
            g = qb * W
            oacc = [oaccp.tile([128, W], F32, tag="oacc",
                               name=f"oa{m}{qb}{hh}") for hh in range(2)]

            def emit_scores(ki):
                qcs = 0 if ki < 4 * qb else 128 * (ki - 4 * qb)
                spt = ps.tile([128, 1024], F32, tag="ps", name="spt")
                for hh in range(2):
                    r0 = DK * hh
                    nc.tensor.matmul(
                        spt[:, W * hh + qcs:W * hh + W],
                        kT[m][r0:r0 + DK, 128 * ki:128 * ki + 128],
                        qT[m][r0:r0 + DK, g + qcs:g + W],
                        start=True, stop=True)
                et = exppool.tile([128, 1024], BF16, tag="et", name="et")
                if qcs:
                    esrc = spt[:].rearrange(
                        "p (b c) -> p b c", c=W)[:, :, qcs:W]
                    edst = et[:].rearrange(
                        "p (b c) -> p b c", c=W)[:, :, qcs:W]
                else:
                    esrc, edst = spt[:], et[:]
                nc.scalar.activation(
                    out=edst, in_=esrc,
                    func=mybir.ActivationFunctionType.Exp, scale=SCALE)
                if ki >= 4 * qb:
                    # mask: key-chunk 2ki+1 (rows 64:) vs query-chunk 2ki
                    for hh in range(2):
                        nc.gpsimd.memset(
                            et[DK:128, W * hh + qcs:W * hh + qcs + CHUNK],
                            0.0)
                return et

            def emit_attnv(ki, et):
                qcs = 0 if ki < 4 * qb else 128 * (ki - 4 * qb)
                for hh in range(2):
                    vst = v_sb[ki][:, (2 * m + hh) * 2 * DK:
                                   (2 * m + hh + 1) * 2 * DK]
                    nc.tensor.matmul(
                        oacc[hh][:, qcs:W], vst, et[:, W * hh + qcs:W * hh + W],
                        start=(ki == 0), stop=(ki == kn - 1))

            pend = []
            for ki in range(kn):
                pend.append((ki, emit_scores(ki)))
                if len(pend) > LAG:
                    k0, e0 = pend.pop(0)
                    emit_attnv(k0, e0)
                if inject:
                    inject.pop(0)()
            for k0, e0 in pend:
                emit_attnv(k0, e0)
            while inject:
                inject.pop(0)()
            # softmax norms: denominator rows DK:128 (copy to SBUF first —
            # reciprocal_approx_fast misreads PSUM on HW)
            for hh in range(2):
                dn = rcpool.tile([DK, W], F32, tag="dn", name="dn")
                nc.vector.tensor_copy(
                    out=dn[:], in_=oacc[hh][DK:2 * DK, :])
                rc = rcpool.tile([DK, W], F32, tag="rc", name="rc")
                nc.vector.reciprocal_approx_fast(out=rc[:], in_=dn[:])
                nc.vector.tensor_tensor(
                    out=att[m][DK * hh:DK * hh + DK, g:g + W],
                    in0=oacc[hh][0:DK, :], in1=rc[:],
                    op=mybir.AluOpType.mult)

        # ====== output projection for q-block ======
        def emit_outproj(qb, tis=None):
            for ti in (tis if tis is not None else range(qb * 4, qb * 4 + 4)):
                po = ps.tile([128, W], F32, tag="ps", name="po")
                for m2 in range(2):
                    nc.tensor.matmul(
                        po[:], att[m2][:, ti * 128:(ti + 1) * 128],
                        wout_sb[m2], start=(m2 == 0), stop=(m2 == 1))
                og = opool.tile([128, W], F32, tag="og", name="og")
                nc.vector.tensor_copy(out=og[:], in_=po[:])
                nc.sync.dma_start(
                    out=out_d[ti * 128:(ti + 1) * 128, :], in_=og[:])

        # ====== emission schedule ======
        def prep_pieces(tcn):
            p = []
            for kd in range(KD):
                p.append(lambda tcn=tcn, kd=kd: emit_norm_x(tcn, kd))
            p.append(lambda tcn=tcn: emit_proj(0, tcn))
            p.append(lambda tcn=tcn: emit_proj(1, tcn))
            for ti in range(4 * tcn, 4 * tcn + 4):
                p.append(lambda ti=ti: emit_v(ti))
            return p

        def op_pieces(qb):
            return [lambda ti=ti: emit_outproj(qb, tis=[ti])
                    for ti in range(qb * 4, qb * 4 + 4)]

        nop = lambda: None

        def pad(pieces, n):
            return pieces + [nop] * max(0, n - len(pieces))

        for tb in range(NQB):
            emit_stats(tb)
        for piece in prep_pieces(0):
            piece()
        # wave 0: att qb0 (4+4 ki slots) + prep(1)
        w1 = prep_pieces(1)
        emit_att(0, 0, inject=pad(w1[:5], 4))
        emit_att(1, 0, inject=pad(w1[5:], 4))
        # wave 1: att qb1 (8+8 slots) + prep(2) + outproj(0)
        w2 = prep_pieces(2)
        op0 = op_pieces(0)
        emit_att(0, 1, inject=pad(w2[:6] + op0[:2], 8))
        emit_att(1, 1, inject=pad(w2[6:] + op0[2:], 8))
        # wave 2: att qb2 (12+12 slots) + prep(3) + outproj(1)
        w3 = prep_pieces(3)
        op1 = op_pieces(1)
        emit_att(0, 2, inject=pad(w3[:6] + op1[:2], 12))
        emit_att(1, 2, inject=pad(w3[6:] + op1[2:], 12))
        # wave 3: att qb3 (16+16 slots) + outproj(2), then outproj(3)
        op2 = op_pieces(2)
        emit_att(0, 3, inject=pad(op2[:2], 16))
        emit_att(1, 3, inject=pad(op2[2:], 16))
        emit_outproj(3)


_CACHED_NC = None


def _get_nc():
    global _CACHED_NC
    if _CACHED_NC is None:
        _CACHED_NC = _build_program()
    return _CACHED_NC


def make_in_maps(x, pos_enc, mask, ln_w, ln_b, Wq, bq, Wk, bk, Wv, bv,
                 Wpos, Wout, bout):
    f32, bf = np.float32, ml_dtypes.bfloat16
    x = np.asarray(x, f32)
    pos_enc = np.asarray(pos_enc, f32)
    ln_w = np.asarray(ln_w, f32)
    ln_b = np.asarray(ln_b, f32)
    Wq, bq = np.asarray(Wq, f32), np.asarray(bq, f32)
    Wk, bk = np.asarray(Wk, f32), np.asarray(bk, f32)
    Wv, bv = np.asarray(Wv, f32), np.asarray(bv, f32)
    Wpos = np.asarray(Wpos, f32)
    Wout = np.asarray(Wout, f32)

    # Fold the LayerNorm affine into the projections (exact rewrite).
    lw = ln_w[:, None]
    Wq_f, bq_f = Wq * lw, bq + ln_b @ Wq
    Wk_f, bk_f = Wk * lw, bk + ln_b @ Wk
    Wv_f, bv_f = Wv * lw, bv + ln_b @ Wv

    # Host-side pos projection (+ bk), shipped transposed.
    pk_full = pos_enc[0] @ Wpos  # [T, D]

    in_maps = []
    for c in range(NCORES):
        b, g = divmod(c, 2)
        hs = slice(g * F, (g + 1) * F)
        # x transposed, tb-major: [4 tb][512 d][512 t]
        xt = np.ascontiguousarray(
            x[b].T.reshape(D, NQB, W).transpose(1, 0, 2).reshape(T, D))
        in_maps.append({
            "xt": xt.astype(bf),
            "wq": np.ascontiguousarray(Wq_f[:, hs]).astype(bf),
            "wk": np.ascontiguousarray(Wk_f[:, hs]).astype(bf),
            "wv": np.ascontiguousarray(Wv_f[:, hs]).astype(bf),
            "wout": np.ascontiguousarray(Wout[hs, :]).astype(bf),
            "pkT": np.ascontiguousarray(
                (pk_full[:, hs] + bk_f[hs]).T).astype(bf),
            "bq": np.ascontiguousarray(bq_f[hs, None]),
            "bv": np.ascontiguousarray(bv_f[None, hs]),
        })
    return in_maps


def kernel(**inputs):
    in_maps = make_in_maps(**inputs)
    bout = np.asarray(inputs["bout"], np.float32)
    nc = _get_nc()
    res = run_bass_kernel_spmd(nc, in_maps, core_ids=list(range(NCORES)))

    out = np.empty((B, T, D), np.float32)
    for b in range(B):
        out[b] = res.results[2 * b]["out"] + res.results[2 * b + 1]["out"] + bout
    return out


# revision 18
# speedup vs baseline: 1.3619x; 1.0189x over previous
"""ChunkAwareAttention Trainium2 kernel (bf16 datapath), v2.

Model (hardcoded): B=4, T=2048, D=512, H=8, DK=64, CHUNK=64, EPS=1e-5.
  xn = LayerNorm(x) * ln_w + ln_b          (affine folded into W on host)
  q/k/v = heads(xn @ W{q,k,v} + b)         [B,H,T,DK]
  scores = (q @ (k + pk)^T) / sqrt(DK)     (pos term + bk folded into kT)
  chunk-causal mask, softmax, @v, out = concat_heads @ Wout + bout

Sharding over 8 cores: core c -> batch b = c//2, head-group g = c%2
(4 heads = 256 features per core). Host sums the two partials per batch.

v2 design (vs v1):
  - x shipped TRANSPOSED (tb-major [4][512d][512t]); LayerNorm stats via
    PE ones-matmuls producing partition-replicated sums, so no PE
    transposes and no bn_stats chain.  Normalize runs on GpSimd.
  - scores for the two heads of an m-group are row-packed: head A uses
    PE rows 0:63, head B rows 64:127 (tile_position auto-derived from
    base_partition) -> the two matmuls run concurrently.
  - score PSUM tile [128, 1024] = headA 512 q-cols | headB 512 q-cols
    (different banks); ONE exp per k-tile covers both heads.
  - chunk-diagonal masking via GpSimd memset-0 on the exp'd tile
    (replaces rank-1 -1e30 matmuls on PE).
  - softmax denominators via ones-columns in v (free: matmul cost is
    moving-column-bound); reciprocal reads PSUM directly.
  - wavefront schedule: attention q-blocks of 512 per m-group; LN
    normalize / projections / v / out-projection interleave into the
    ACT-bound attention stream to keep PE dense (HAM stays warm).
  - all sqrt before the first exp -> exactly 2 ACT table loads.
"""

import sys

if "/opt/trn_rl_repo" not in sys.path:
    sys.path.insert(0, "/opt/trn_rl_repo")

import math
import numpy as np
import ml_dtypes

import concourse.bass as bass
import concourse.tile as tile
from concourse import bacc, mybir
from concourse.bass_utils import run_bass_kernel_spmd

B, T, D, H = 4, 2048, 512, 8
DK = D // H
CHUNK = 64
EPS = 1e-5
NCORES = 8
HPC = H // 2          # heads per core = 4
F = HPC * DK          # features per core = 256
KD = D // 128         # contraction tiles over D = 4
NT = T // 128         # 128-row tiles over T = 16
F32 = mybir.dt.float32
BF16 = mybir.dt.bfloat16
SCALE = 1.0 / math.sqrt(DK)
LAG = 2
W = 512               # attention q-block width
NQB = T // W          # q-blocks = 4


def _build_program():
    nc = bacc.Bacc(
        "TRN2",
        target_bir_lowering=False,
        debug=False,
        enable_asserts=False,
        num_devices=NCORES,
    )

    # xt: tb-major transposed x: [4 tb][512 d][512 t] flattened.
    xt_d = nc.dram_tensor("xt", [T, D], BF16, kind="ExternalInput").ap()
    wq_d = nc.dram_tensor("wq", [D, F], BF16, kind="ExternalInput").ap()
    wk_d = nc.dram_tensor("wk", [D, F], BF16, kind="ExternalInput").ap()
    wv_d = nc.dram_tensor("wv", [D, F], BF16, kind="ExternalInput").ap()
    wout_d = nc.dram_tensor("wout", [F, D], BF16, kind="ExternalInput").ap()
    pkT_d = nc.dram_tensor("pkT", [F, T], BF16, kind="ExternalInput").ap()
    bq_d = nc.dram_tensor("bq", [F, 1], F32, kind="ExternalInput").ap()
    bv_d = nc.dram_tensor("bv", [1, F], F32, kind="ExternalInput").ap()
    out_d = nc.dram_tensor("out", [T, D], F32, kind="ExternalOutput").ap()

    with tile.TileContext(nc) as tc:
        _emit(nc, tc, xt_d, wq_d, wk_d, wv_d, wout_d, pkT_d,
              bq_d, bv_d, out_d)

    nc.compile()
    return nc


def _emit(nc, tc, xt_d, wq_d, wk_d, wv_d, wout_d, pkT_d, bq_d, bv_d, out_d):
    from contextlib import ExitStack

    ctx = ExitStack()
    with ctx:
        singles = ctx.enter_context(tc.tile_pool(name="singles", bufs=1))
        xsqpool = ctx.enter_context(tc.tile_pool(name="xsq", bufs=2))
        stpool = ctx.enter_context(tc.tile_pool(name="st", bufs=4))
        t1pool = ctx.enter_context(tc.tile_pool(name="t1", bufs=3))
        exppool = ctx.enter_context(tc.tile_pool(name="exp", bufs=4))
        rcpool = ctx.enter_context(tc.tile_pool(name="rc", bufs=4))
        opool = ctx.enter_context(tc.tile_pool(name="ostage", bufs=3))
        # PSUM: ps 2 x [128,1024] (4 banks) + oacc 2 x [128,512] (2) +
        # free 2 x [128,512] (2) = 8 banks.
        ps = ctx.enter_context(tc.tile_pool(name="ps", bufs=2, space="PSUM"))
        oaccp = ctx.enter_context(
            tc.tile_pool(name="oacc", bufs=2, space="PSUM"))
        freep = ctx.enter_context(
            tc.tile_pool(name="free", bufs=2, space="PSUM"))

        # ---- input DMAs (xt on sync queue, tb0 first; weights on gpsimd) --
        xtb = []
        for tb in range(NQB):
            t = singles.tile([128, KD * W], BF16, tag=f"xt{tb}",
                             name=f"xt{tb}")
            src = bass.AP(tensor=xt_d.tensor,
                          offset=xt_d.offset + tb * W * D,
                          ap=[[W, 128], [128 * W, KD], [1, W]])
            dst = t[:].rearrange("p (b c) -> p b c", c=W)
            q = nc.sync if tb % 2 == 0 else nc.gpsimd
            q.dma_start(out=dst, in_=src)
            xtb.append(t)

        def load_mono(dram, nblk, width, nm):
            t = singles.tile([128, nblk * width], BF16, tag=nm, name=nm)
            src = bass.AP(tensor=dram.tensor, offset=dram.offset,
                          ap=[[width, 128], [128 * width, nblk], [1, width]])
            dst = t[:].rearrange("p (b c) -> p b c", c=width)
            nc.gpsimd.dma_start(out=dst, in_=src)
            return t
        wq_t = load_mono(wq_d, KD, F, "wqall")
        wk_t = load_mono(wk_d, KD, F, "wkall")
        wv_t = load_mono(wv_d, KD, F, "wvall")
        wq_sb = [wq_t[:, kd * F:(kd + 1) * F] for kd in range(KD)]
        wk_sb = [wk_t[:, kd * F:(kd + 1) * F] for kd in range(KD)]
        wv_sb = [wv_t[:, kd * F:(kd + 1) * F] for kd in range(KD)]
        pkT_t = load_mono(pkT_d, 2, T, "pkTall")
        pkT_sb = [pkT_t[:, m * T:(m + 1) * T] for m in range(2)]
        wout_t = load_mono(wout_d, 2, D, "woutall")
        wout_sb = [wout_t[:, m * D:(m + 1) * D] for m in range(2)]
        bq_t = singles.tile([128, 2], F32, tag="bqall", name="bq_t")
        nc.sync.dma_start(out=bq_t[:], in_=bass.AP(
            tensor=bq_d.tensor, offset=bq_d.offset, ap=[[1, 128], [128, 2]]))
        bq_sb = [bq_t[:, m:m + 1] for m in range(2)]
        bv_sb = singles.tile([128, F], F32)
        nc.gpsimd.dma_start(
            out=bv_sb[:],
            in_=bass.AP(tensor=bv_d.tensor, offset=bv_d.offset,
                        ap=[[0, 128], [1, F]]))

        # ---- consts ----
        ones_t = singles.tile([128, 128], BF16, tag="ones")
        nc.vector.memset(ones_t[:], 1.0)
        eps_t = singles.tile([128, 1], F32)
        nc.vector.memset(eps_t, EPS)

        # ---- stats staging (partition-replicated) ----
        mu_all = singles.tile([128, T], BF16, tag="mu", name="mu_all")
        rstd_all = singles.tile([128, T], BF16, tag="rstd", name="rstd_all")
        murstd_all = singles.tile([128, T], BF16, tag="murstd",
                                  name="murstd_all")

        # ---- big resident activations (bf16) ----
        xnT = singles.tile([128, KD * T], BF16, tag="xnT", name="xnT")
        qT = [singles.tile([128, T], BF16, tag=f"qT{m}", name=f"qT{m}")
              for m in range(2)]
        kT = [singles.tile([128, T], BF16, tag=f"kT{m}", name=f"kT{m}")
              for m in range(2)]
        # V natural layout, per head [V_h(64) | ones(64)]: ones columns give
        # the softmax denominator on PSUM rows 64:128 of the attnv matmul.
        v_sb = [singles.tile([128, HPC * (2 * DK)], BF16, tag=f"v{ti}",
                             name=f"v{ti}") for ti in range(NT)]
        for ti in range(NT):
            nc.vector.memset(v_sb[ti][:], 1.0)  # pre-fill ones columns
        att = [singles.tile([128, T], BF16, tag=f"att{m}", name=f"att{m}")
               for m in range(2)]

        def xnT_c(kd, lo, hi):
            return xnT[:, kd * T + lo:kd * T + hi]

        # ====== LayerNorm stats for one 512-col t-block ======
        def emit_stats(tb):
            xsq = xsqpool.tile([128, KD * W], BF16)
            nc.vector.tensor_tensor(
                out=xsq[:], in0=xtb[tb][:], in1=xtb[tb][:],
                op=mybir.AluOpType.mult)
            psum = freep.tile([128, W], F32, tag="free", name="ps_sum")
            pssq = freep.tile([128, W], F32, tag="free", name="ps_sq")
            for kd in range(KD):
                nc.tensor.matmul(
                    psum[:], ones_t[:], xtb[tb][:, kd * W:(kd + 1) * W],
                    start=(kd == 0), stop=(kd == KD - 1))
            for kd in range(KD):
                nc.tensor.matmul(
                    pssq[:], ones_t[:], xsq[:, kd * W:(kd + 1) * W],
                    start=(kd == 0), stop=(kd == KD - 1))
            tsl = slice(tb * W, (tb + 1) * W)
            nc.vector.tensor_scalar_mul(
                out=mu_all[:, tsl], in0=psum[:], scalar1=1.0 / D)
            msq = stpool.tile([128, W], F32)
            nc.vector.tensor_scalar_mul(
                out=msq[:], in0=pssq[:], scalar1=1.0 / D)
            mumu = stpool.tile([128, W], F32)
            nc.vector.tensor_tensor(
                out=mumu[:], in0=mu_all[:, tsl], in1=mu_all[:, tsl],
                op=mybir.AluOpType.mult)
            var = stpool.tile([128, W], F32)
            nc.vector.tensor_tensor(
                out=var[:], in0=msq[:], in1=mumu[:],
                op=mybir.AluOpType.subtract)
            sd = stpool.tile([128, W], F32)
            nc.scalar.activation(
                out=sd[:], in_=var[:],
                func=mybir.ActivationFunctionType.Sqrt, bias=eps_t[:],
                scale=1.0)
            rstdf = stpool.tile([128, W], F32)
            nc.vector.reciprocal_approx_fast(out=rstdf[:], in_=sd[:])
            nc.vector.tensor_copy(out=rstd_all[:, tsl], in_=rstdf[:])
            nc.vector.tensor_tensor(
                out=murstd_all[:, tsl], in0=mu_all[:, tsl],
                in1=rstd_all[:, tsl], op=mybir.AluOpType.mult)

        # ====== normalize one (kd, tcn) tile (DVE, bf16 2x mode) ======
        def emit_norm_x(tcn, kd):
            tsl = slice(tcn * W, (tcn + 1) * W)
            t1 = t1pool.tile([128, W], BF16)
            nc.vector.tensor_tensor(
                out=t1[:], in0=xtb[tcn][:, kd * W:(kd + 1) * W],
                in1=rstd_all[:, tsl], op=mybir.AluOpType.mult)
            nc.vector.tensor_tensor(
                out=xnT_c(kd, tcn * W, tcn * W + W), in0=t1[:],
                in1=murstd_all[:, tsl], op=mybir.AluOpType.subtract)

        # ====== q/k projections for (m, tcn) ======
        def emit_proj(m, tcn):
            tsl = slice(tcn * W, (tcn + 1) * W)
            msl = slice(m * 128, (m + 1) * 128)
            pq = freep.tile([128, W], F32, tag="free", name="pq")
            for kd in range(KD):
                nc.tensor.matmul(
                    pq[:], wq_sb[kd][:, msl], xnT_c(kd, tcn * W, tcn * W + W),
                    start=(kd == 0), stop=(kd == KD - 1))
            nc.vector.tensor_scalar_add(
                out=qT[m][:, tsl], in0=pq[:], scalar1=bq_sb[m])
            pk = freep.tile([128, W], F32, tag="free", name="pk")
            for kd in range(KD):
                nc.tensor.matmul(
                    pk[:], wk_sb[kd][:, msl], xnT_c(kd, tcn * W, tcn * W + W),
                    start=(kd == 0), stop=(kd == KD - 1))
            # kT = pk + pkT  (host pos projection, bk folded in)
            nc.vector.tensor_tensor(
                out=kT[m][:, tsl], in0=pk[:], in1=pkT_sb[m][:, tsl],
                op=mybir.AluOpType.add)

        # ====== v projection for one 128-row tile ======
        def emit_v(ti):
            pv = freep.tile([128, F], F32, tag="free", name="pv")
            for kd in range(KD):
                nc.tensor.matmul(
                    pv[:], xnT_c(kd, ti * 128, ti * 128 + 128), wv_sb[kd],
                    start=(kd == 0), stop=(kd == KD - 1))
            vt = v_sb[ti]
            dst = vt[:].rearrange("p (h c) -> p h c", h=HPC)[:, :, 0:DK]
            srcv = pv[:].rearrange("p (h c) -> p h c", c=DK)
            bvb = bv_sb[:].rearrange("p (h c) -> p h c", c=DK)
            nc.vector.tensor_tensor(
                out=dst, in0=srcv, in1=bvb, op=mybir.AluOpType.add)

        # ====== attention: one global pipeline over (qb, m, ki) units ======
        # scores+exp run LAG units ahead of attnv; block N+1's scores
        # overlap block N's attnv tail + softmax norms (no flush stall).
        def emit_scores(m, qb, ki):
            g = qb * W
            qcs = 0 if ki < 4 * qb else 128 * (ki - 4 * qb)
            spt = ps.tile([128, 1024], F32, tag="ps", name="spt")
            for hh in range(2):
                r0 = DK * hh
                nc.tensor.matmul(
                    spt[:, W * hh + qcs:W * hh + W],
                    kT[m][r0:r0 + DK, 128 * ki:128 * ki + 128],
                    qT[m][r0:r0 + DK, g + qcs:g + W],
                    start=True, stop=True)
            et = exppool.tile([128, 1024], BF16, tag="et", name="et")
            if qcs:
                esrc = spt[:].rearrange("p (b c) -> p b c", c=W)[:, :, qcs:W]
                edst = et[:].rearrange("p (b c) -> p b c", c=W)[:, :, qcs:W]
            else:
                esrc, edst = spt[:], et[:]
            nc.scalar.activation(
                out=edst, in_=esrc,
                func=mybir.ActivationFunctionType.Exp, scale=SCALE)
            if ki >= 4 * qb:
                # mask: key-chunk 2ki+1 (rows 64:) vs query-chunk 2ki
                for hh in range(2):
                    nc.gpsimd.memset(
                        et[DK:128, W * hh + qcs:W * hh + qcs + CHUNK], 0.0)
            return et

        def emit_attention(inject_map):
            oaccs = {}
            pend = []

            def do_attnv(m, qb, ki, et):
                kn = 4 * qb + 4
                if ki == 0:
                    oaccs[(m, qb)] = [
                        oaccp.tile([128, W], F32, tag="oacc",
                                   name=f"oa{m}{qb}{hh}") for hh in range(2)]
                oacc = oaccs[(m, qb)]
                qcs = 0 if ki < 4 * qb else 128 * (ki - 4 * qb)
                for hh in range(2):
                    vst = v_sb[ki][:, (2 * m + hh) * 2 * DK:
                                   (2 * m + hh + 1) * 2 * DK]
                    nc.tensor.matmul(
                        oacc[hh][:, qcs:W],
                        vst, et[:, W * hh + qcs:W * hh + W],
                        start=(ki == 0), stop=(ki == kn - 1))
                if ki == kn - 1:
                    # softmax norms (denominator copy to SBUF first —
                    # reciprocal_approx_fast misreads PSUM on HW)
                    for hh in range(2):
                        dn = rcpool.tile([DK, W], F32, tag="dn", name="dn")
                        nc.vector.tensor_copy(
                            out=dn[:], in_=oacc[hh][DK:2 * DK, :])
                        rc = rcpool.tile([DK, W], F32, tag="rc", name="rc")
                        nc.vector.reciprocal_approx_fast(out=rc[:], in_=dn[:])
                        nc.vector.tensor_tensor(
                            out=att[m][DK * hh:DK * hh + DK,
                                       qb * W:qb * W + W],
                            in0=oacc[hh][0:DK, :], in1=rc[:],
                            op=mybir.AluOpType.mult)
                    del oaccs[(m, qb)]

            idx = 0
            for qb in range(NQB):
                for m in range(2):
                    for ki in range(4 * qb + 4):
                        et = emit_scores(m, qb, ki)
                        pend.append((m, qb, ki, et))
                        if len(pend) > LAG:
                            do_attnv(*pend.pop(0))
                        for piece in inject_map.get(idx, []):
                            piece()
                        idx += 1
            for u in pend:
                do_attnv(*u)

        # ====== output projection for q-block ======
        def emit_outproj(qb, tis=None):
            for ti in (tis if tis is not None else range(qb * 4, qb * 4 + 4)):
                po = ps.tile([128, W], F32, tag="ps", name="po")
                for m2 in range(2):
                    nc.tensor.matmul(
                        po[:], att[m2][:, ti * 128:(ti + 1) * 128],
                        wout_sb[m2], start=(m2 == 0), stop=(m2 == 1))
                og = opool.tile([128, W], F32, tag="og", name="og")
                nc.vector.tensor_copy(out=og[:], in_=po[:])
                nc.sync.dma_start(
                    out=out_d[ti * 128:(ti + 1) * 128, :], in_=og[:])

        # ====== emission schedule ======
        # units: qb0 -> idx 0-7, qb1 -> 8-23, qb2 -> 24-47, qb3 -> 48-79
        def prep_pieces(tcn):
            return [
                lambda: (emit_norm_x(tcn, 0), emit_norm_x(tcn, 1)),
                lambda: (emit_norm_x(tcn, 2), emit_norm_x(tcn, 3)),
                lambda: emit_proj(0, tcn),
                lambda: emit_proj(1, tcn),
            ] + [lambda ti=ti: emit_v(ti)
                 for ti in range(4 * tcn, 4 * tcn + 4)]

        def op_pieces(qb):
            return [lambda ti=ti: emit_outproj(qb, tis=[ti])
                    for ti in range(qb * 4, qb * 4 + 4)]

        for tb in range(NQB):
            emit_stats(tb)
        for piece in prep_pieces(0):
            piece()

        inject_map = {}
        for i, piece in enumerate(prep_pieces(1)):
            inject_map.setdefault(0 + i, []).append(piece)     # during qb0
        for i, piece in enumerate(prep_pieces(2)):
            inject_map.setdefault(8 + i, []).append(piece)     # during qb1 m0
        for i, piece in enumerate(op_pieces(0)):
            inject_map.setdefault(18 + i, []).append(piece)    # during qb1 m1
        for i, piece in enumerate(prep_pieces(3)):
            inject_map.setdefault(24 + i, []).append(piece)    # during qb2 m0
        for i, piece in enumerate(op_pieces(1)):
            inject_map.setdefault(38 + i, []).append(piece)    # during qb2 m1
        for i, piece in enumerate(op_pieces(2)):
            inject_map.setdefault(52 + i, []).append(piece)    # during qb3
        emit_attention(inject_map)
        emit_outproj(3)


_CACHED_NC = None


def _get_nc():
    global _CACHED_NC
    if _CACHED_NC is None:
        _CACHED_NC = _build_program()
    return _CACHED_NC


def make_in_maps(x, pos_enc, mask, ln_w, ln_b, Wq, bq, Wk, bk, Wv, bv,
                 Wpos, Wout, bout):
    f32, bf = np.float32, ml_dtypes.bfloat16
    x = np.asarray(x, f32)
    pos_enc = np.asarray(pos_enc, f32)
    ln_w = np.asarray(ln_w, f32)
    ln_b = np.asarray(ln_b, f32)
    Wq, bq = np.asarray(Wq, f32), np.asarray(bq, f32)
    Wk, bk = np.asarray(Wk, f32), np.asarray(bk, f32)
    Wv, bv = np.asarray(Wv, f32), np.asarray(bv, f32)
    Wpos = np.asarray(Wpos, f32)
    Wout = np.asarray(Wout, f32)

    # Fold the LayerNorm affine into the projections (exact rewrite).
    lw = ln_w[:, None]
    Wq_f, bq_f = Wq * lw, bq + ln_b @ Wq
    Wk_f, bk_f = Wk * lw, bk + ln_b @ Wk
    Wv_f, bv_f = Wv * lw, bv + ln_b @ Wv

    # Host-side pos projection (+ bk), shipped transposed.
    pk_full = pos_enc[0] @ Wpos  # [T, D]

    in_maps = []
    for c in range(NCORES):
        b, g = divmod(c, 2)
        hs = slice(g * F, (g + 1) * F)
        # x transposed, tb-major: [4 tb][512 d][512 t]
        xt = np.ascontiguousarray(
            x[b].T.reshape(D, NQB, W).transpose(1, 0, 2).reshape(T, D))
        in_maps.append({
            "xt": xt.astype(bf),
            "wq": np.ascontiguousarray(Wq_f[:, hs]).astype(bf),
            "wk": np.ascontiguousarray(Wk_f[:, hs]).astype(bf),
            "wv": np.ascontiguousarray(Wv_f[:, hs]).astype(bf),
            "wout": np.ascontiguousarray(Wout[hs, :]).astype(bf),
            "pkT": np.ascontiguousarray(
                (pk_full[:, hs] + bk_f[hs]).T).astype(bf),
            "bq": np.ascontiguousarray(bq_f[hs, None]),
            "bv": np.ascontiguousarray(bv_f[None, hs]),
        })
    return in_maps


def kernel(**inputs):
    in_maps = make_in_maps(**inputs)
    bout = np.asarray(inputs["bout"], np.float32)
    nc = _get_nc()
    res = run_bass_kernel_spmd(nc, in_maps, core_ids=list(range(NCORES)))

    out = np.empty((B, T, D), np.float32)
    for b in range(B):
        out[b] = res.results[2 * b]["out"] + res.results[2 * b + 1]["out"] + bout
    return out
